# revision 16
# baseline (speedup 1.0000x reference)
"""
Trainium2 distributed kernel for causal multi-head attention
(nn_Attention: B=4, S=2048, D=768, H=4, DH=192).

Sharding: 16 (batch, head) units across 8 cores = 1 batch x 2 heads per
core.  Every core runs an identical graph (SPMD) on its own shard:

  xT   [768, 2048]  x[b].T                     (bf16)
  wqT  [768, 384]   Wq[cols,:].T               (bf16)
  wkT  [768, 384]   Wk[cols,:].T               (bf16)
  wvT  [768, 384]   Wv[cols,:].T               (bf16)
  woS  [384, 768]   Wo[:,cols].T               (bf16)
  out  [2048, 768]  partial output (sum of this core's 2 heads)  (f32)

The final output is out[b] = partial[2b] + partial[2b+1] summed on the
host (the unshard step for output-partial sharding).  No on-device
collectives are required and the per-core causal work is identical, so
the 8 cores are perfectly load balanced.

Device algorithm (all matmuls bf16 with f32 PSUM accumulation):
  QT = (x Wq_s.T).T and KT likewise, stored [dh-chunks, seq]
  V stored naturally [seq, dh] with an appended ones column, so the
  AV matmul also produces the softmax denominators (rows of ones.T @ A).
  Scores are computed transposed, S.T[k, q] = KT.T @ QT, so that the
  exp'd attention tiles feed the AV matmul directly as the moving
  operand with no transposes anywhere.  Softmax uses no max-subtraction
  (logits are O(1) by construction), the causal mask is additive on the
  diagonal blocks only; fully-masked blocks are never computed.
"""

import math
import os
import sys

import numpy as np

for _p in ("/opt/trn_rl_repo",):
    if _p not in sys.path and os.path.isdir(_p):
        sys.path.insert(0, _p)

import ml_dtypes  # noqa: E402

B, S, D, H = 4, 2048, 768, 4
DH = D // H  # 192
HPC = 2  # heads per core
HD = HPC * DH  # 384 head dims handled per core
P = 128
KD = D // P  # 6 contraction chunks over D
QB = 512  # query block (matmul moving dim)
NQ = S // QB  # 4
KB = 128  # key block (psum partition dim)
NK = S // KB  # 16
MS = S // P  # 16 seq chunks
SCALE = 1.0 / math.sqrt(DH)
MASK_NEG = -1e9

# per-head lhsT chunk layout for the 384 head dims: (offset, size)
CHUNKS = [(0, 128), (128, 64), (192, 128), (320, 64)]

_CACHED = {}


def build_nc():
    import concourse.mybir as mybir
    from concourse import bacc
    from concourse import tile

    fp32 = mybir.dt.float32
    f32r = mybir.dt.float32r
    bf16 = mybir.dt.bfloat16
    Exp = mybir.ActivationFunctionType.Exp

    nc = bacc.Bacc(None, target_bir_lowering=False)

    xT = nc.declare_dram_parameter("xT", [D, S], bf16, isOutput=False)
    wqT = nc.declare_dram_parameter("wqT", [D, HD], bf16, isOutput=False)
    wkT = nc.declare_dram_parameter("wkT", [D, HD], bf16, isOutput=False)
    wvT = nc.declare_dram_parameter("wvT", [D, HD], bf16, isOutput=False)
    woS = nc.declare_dram_parameter("woS", [HD, D], bf16, isOutput=False)
    out = nc.declare_dram_parameter("out", [S, D], fp32, isOutput=True)

    with tile.TileContext(nc) as tc:
        with (
            tc.tile_pool(name="const", bufs=1) as const,
            tc.tile_pool(name="atp", bufs=2) as atp,
            tc.tile_pool(name="ost", bufs=3) as ostp,
            tc.tile_pool(name="rcp", bufs=2) as rcp,
            tc.tile_pool(name="pps", bufs=2, space="PSUM") as pps,
            tc.tile_pool(name="scps", bufs=2, space="PSUM") as scps,
            tc.tile_pool(name="avps", bufs=1, space="PSUM") as avps,
        ):
            # ---- persistent SBUF tensors ----
            xT_sb = const.tile([P, KD, S], bf16, tag="xT_sb")
            wq_sb = const.tile([P, KD, HD], bf16, tag="wq_sb")
            wk_sb = const.tile([P, KD, HD], bf16, tag="wk_sb")
            wv_sb = const.tile([P, KD, HD], bf16, tag="wv_sb")
            wo_sb = const.tile([P, 4, D], bf16, tag="wo_sb")
            qt_sb = const.tile([P, 4, S], bf16, tag="qt_sb")
            kt_sb = const.tile([P, 4, S], bf16, tag="kt_sb")
            # V with ones column: [k-part, k-chunk, head, 193]
            v_sb = const.tile([P, NK, HPC, DH + 1], bf16, tag="v_sb")
            pt_sb = const.tile([P, 4, S], bf16, tag="pt_sb")
            masks = const.tile([P, 4, QB], fp32, tag="masks")
            ones1 = const.tile([1, P], bf16, tag="ones1")
            nc.vector.memset(ones1[:], 1.0)

            # ---- input DMAs ----
            nc.sync.dma_start(
                xT_sb[:], xT.rearrange("(ko ki) s -> ki ko s", ki=P)
            )
            nc.sync.dma_start(
                wq_sb[:], wqT.rearrange("(ko ki) j -> ki ko j", ki=P)
            )
            nc.sync.dma_start(
                wk_sb[:], wkT.rearrange("(ko ki) j -> ki ko j", ki=P)
            )
            nc.sync.dma_start(
                wv_sb[:], wvT.rearrange("(ko ki) j -> ki ko j", ki=P)
            )
            for c, (off, sz) in enumerate(CHUNKS):
                nc.sync.dma_start(wo_sb[0:sz, c, :], woS[off : off + sz, :])

            # ---- causal masks for the 4 diagonal sub-blocks ----
            # valid (keep 0) iff q_local >= 128*d + k_local
            for d in range(4):
                nc.vector.memset(masks[:, d, :], 0.0)
                nc.gpsimd.affine_select(
                    out=masks[:, d, :],
                    in_=masks[:, d, :],
                    compare_op=mybir.AluOpType.is_ge,
                    fill=MASK_NEG,
                    base=-128 * d,
                    pattern=[[1, QB]],
                    channel_multiplier=-1,
                )

            # ---- Q/K projections (output transposed: [head-dim, seq]) ----
            for w_sb, o_sb in ((wq_sb, qt_sb), (wk_sb, kt_sb)):
                for c, (off, sz) in enumerate(CHUNKS):
                    for nt in range(NQ):
                        ps = pps.tile([P, QB], fp32, tag="proj_ps")
                        for k in range(KD):
                            nc.tensor.matmul(
                                ps[0:sz],
                                lhsT=w_sb[:, k, off : off + sz],
                                rhs=xT_sb[:, k, nt * QB : (nt + 1) * QB],
                                start=(k == 0),
                                stop=(k == KD - 1),
                            )
                        nc.vector.tensor_copy(
                            o_sb[0:sz, c, nt * QB : (nt + 1) * QB], ps[0:sz]
                        )

            # ---- V projection (natural layout) + ones column ----
            for m in range(MS):
                ps = pps.tile([P, QB], fp32, tag="proj_ps")
                for k in range(KD):
                    nc.tensor.matmul(
                        ps[:, 0:HD],
                        lhsT=xT_sb[:, k, m * P : (m + 1) * P],
                        rhs=wv_sb[:, k, :],
                        start=(k == 0),
                        stop=(k == KD - 1),
                    )
                nc.vector.tensor_copy(
                    v_sb[:, m, :, 0:DH],
                    ps[:, 0:HD].rearrange("p (h d) -> p h d", h=HPC),
                )
                nc.vector.memset(v_sb[:, m, :, DH : DH + 1], 1.0)

            # ---- attention + output projection, per q-block ----
            for qj in range(NQ):
                qsl = slice(qj * QB, (qj + 1) * QB)
                nk = 4 * qj + 4  # number of live key blocks (causal)
                for h in range(HPC):
                    at = atp.tile([P, NK, QB], bf16, tag="at")
                    for ki in range(nk):
                        ps = scps.tile([P, QB], fp32, tag="sc_ps")
                        ksl = slice(ki * KB, (ki + 1) * KB)
                        nc.tensor.matmul(
                            ps,
                            lhsT=kt_sb[0:128, 2 * h, ksl],
                            rhs=qt_sb[0:128, 2 * h, qsl],
                            start=True,
                            stop=False,
                        )
                        nc.tensor.matmul(
                            ps,
                            lhsT=kt_sb[0:64, 2 * h + 1, ksl],
                            rhs=qt_sb[0:64, 2 * h + 1, qsl],
                            start=False,
                            stop=True,
                        )
                        d = ki - 4 * qj
                        if d >= 0:
                            nc.vector.tensor_add(ps, ps, masks[:, d, :])
                        nc.scalar.activation(
                            at[:, ki, :], ps, Exp, scale=SCALE
                        )
                    ps0 = avps.tile([P, QB], fp32, tag="av0_ps")
                    ps1 = avps.tile([P, QB], fp32, tag="av1_ps")
                    for ki in range(nk):
                        nc.tensor.matmul(
                            ps0,
                            lhsT=v_sb[:, ki, h, 0:128],
                            rhs=at[:, ki, :],
                            start=(ki == 0),
                            stop=(ki == nk - 1),
                        )
                        nc.tensor.matmul(
                            ps1[0:65],
                            lhsT=v_sb[:, ki, h, 128 : DH + 1],
                            rhs=at[:, ki, :],
                            start=(ki == 0),
                            stop=(ki == nk - 1),
                        )
                    rc = rcp.tile([1, QB], bf16, tag="rc")
                    rcb = rcp.tile([P, QB], fp32, tag="rcb")
                    with nc.allow_low_precision(
                        reason="bf16 recip feeds broadcast matmul; 0.4% scale ok"
                    ):
                        nc.vector.reciprocal(rc, ps1[64:65, :])
                    # broadcast rc to 128 partitions: ones1.T @ rc (K=1 matmul)
                    rcb_ps = scps.tile([P, QB], fp32, tag="sc_ps")
                    nc.tensor.matmul(
                        rcb_ps, lhsT=ones1[:], rhs=rc[:], start=True, stop=True
                    )
                    nc.scalar.copy(rcb[:], rcb_ps)
                    nc.vector.tensor_mul(
                        pt_sb[0:128, 2 * h, qsl], ps0, rcb[:]
                    )
                    nc.vector.tensor_mul(
                        pt_sb[0:64, 2 * h + 1, qsl], ps1[0:64], rcb[0:64]
                    )
                # output projection for the 4 seq chunks of this q block
                for mi in range(4):
                    m = qj * 4 + mi
                    ost = ostp.tile([P, D], fp32, tag="ost")
                    for n in range(2):
                        ps = pps.tile([P, QB], fp32, tag="op_ps")
                        for c, (off, sz) in enumerate(CHUNKS):
                            nc.tensor.matmul(
                                ps[:, 0:384],
                                lhsT=pt_sb[0:sz, c, m * P : (m + 1) * P],
                                rhs=wo_sb[0:sz, c, n * 384 : (n + 1) * 384],
                                start=(c == 0),
                                stop=(c == 3),
                            )
                        nc.vector.tensor_copy(
                            ost[:, n * 384 : (n + 1) * 384], ps[:, 0:384]
                        )
                    nc.sync.dma_start(out[m * P : (m + 1) * P, :], ost[:])

    nc.compile()
    return nc


def _shard_inputs(x, Wq, Wk, Wv, Wo):
    bf = ml_dtypes.bfloat16
    in_maps = []
    for core in range(8):
        b, hp = core // 2, core % 2
        cols = slice(hp * HD, (hp + 1) * HD)
        in_maps.append(
            {
                "xT": np.ascontiguousarray(x[b].T).astype(bf),
                "wqT": np.ascontiguousarray(Wq[cols, :].T).astype(bf),
                "wkT": np.ascontiguousarray(Wk[cols, :].T).astype(bf),
                "wvT": np.ascontiguousarray(Wv[cols, :].T).astype(bf),
                "woS": np.ascontiguousarray(Wo[:, cols].T).astype(bf),
            }
        )
    return in_maps


def _run(inputs, trace=False, **kw):
    from concourse.bass_utils import run_bass_kernel_spmd

    if "nc" not in _CACHED:
        _CACHED["nc"] = build_nc()
    nc = _CACHED["nc"]
    in_maps = _shard_inputs(
        np.asarray(inputs["x"], np.float32),
        np.asarray(inputs["Wq"], np.float32),
        np.asarray(inputs["Wk"], np.float32),
        np.asarray(inputs["Wv"], np.float32),
        np.asarray(inputs["Wo"], np.float32),
    )
    res = run_bass_kernel_spmd(
        nc, in_maps, core_ids=list(range(8)), trace=trace, **kw
    )
    parts = [np.asarray(r["out"], np.float32) for r in res.results]
    full = np.empty((B, S, D), np.float32)
    for b in range(B):
        full[b] = parts[2 * b] + parts[2 * b + 1]
    return full, res


def kernel(**inputs) -> np.ndarray:
    full, _ = _run(inputs, trace=False)
    return full


# revision 32
# speedup vs baseline: 1.1548x; 1.1548x over previous
"""
Trainium2 distributed kernel for causal multi-head attention
(nn_Attention: B=4, S=2048, D=768, H=4, DH=192).

Sharding: 16 (batch, head) units across 8 cores = 1 batch x 2 heads per
core.  Every core runs an identical graph (SPMD) on its own shard; the
host sums core pairs (the unshard for output-partial sharding).  No
on-device collectives, perfectly balanced causal work.

Device algorithm (bf16 matmuls, f32 PSUM accumulation):
  QT/KT stored transposed [head-dim planes, seq]; the two heads' upper
  64 head-dims share one 128-partition plane (host permutes weight
  columns to match), so every projection matmul contracts a full 128
  partitions and the two 64-row score matmuls run in disjoint PE row
  groups (concurrent).  V is stored naturally [seq, dh] with per-head
  ones columns so the AV matmul also emits softmax denominators.
  Scores are computed transposed, S.T[k, q] = KT.T @ QT, so exp'd
  attention tiles feed AV directly as the moving operand — no
  transposes anywhere.  Softmax skips max-subtraction (logits are O(1)
  by construction); the causal mask is additive on diagonal blocks
  only; fully-masked blocks are never computed.
"""

import math
import os
import sys

import numpy as np

for _p in ("/opt/trn_rl_repo",):
    if _p not in sys.path and os.path.isdir(_p):
        sys.path.insert(0, _p)

import ml_dtypes  # noqa: E402

B, S, D, H = 4, 2048, 768, 4
DH = D // H  # 192
HPC = 2  # heads per core
HD = HPC * DH  # 384 head dims per core
P = 128
KD = D // P  # 6 contraction chunks over D
QB = 512  # query block (matmul moving dim)
NQ = S // QB  # 4
KB = 128  # key block (psum partition dim)
NK = S // KB  # 16
MS = S // P  # 16 seq chunks
SCALE = 1.0 / math.sqrt(DH)
MASK_NEG = -1e9

# host-side column permutation for Wq/Wk (and row perm for Wo):
# planes = [h0 dh0:128 | h1 dh0:128 | h0 dh128:192, h1 dh128:192]
PQ = np.r_[0:128, 192:320, 128:192, 320:384]
# for Wv: [h0 dh0:192 | h1 dh128:192 | h1 dh0:128] so that the SBUF V
# tile [.. h0dh(192), ones0, ones1, h1dh128:192(64), h1dh0:128(128)]
# fills with two contiguous copies
PV = np.r_[0:192, 320:384, 192:320]

_CACHED = {}


def build_nc():
    import concourse.mybir as mybir
    from concourse import bacc
    from concourse import tile

    fp32 = mybir.dt.float32
    bf16 = mybir.dt.bfloat16
    Exp = mybir.ActivationFunctionType.Exp

    nc = bacc.Bacc(None, target_bir_lowering=False)

    xT = nc.declare_dram_parameter("xT", [D, S], bf16, isOutput=False)
    wqT = nc.declare_dram_parameter("wqT", [D, HD], bf16, isOutput=False)
    wkT = nc.declare_dram_parameter("wkT", [D, HD], bf16, isOutput=False)
    wvT = nc.declare_dram_parameter("wvT", [D, HD], bf16, isOutput=False)
    woS = nc.declare_dram_parameter("woS", [HD, D], bf16, isOutput=False)
    out = nc.declare_dram_parameter("out", [S, D], fp32, isOutput=True)

    # V sbuf free-layout offsets
    V_H0C0 = slice(0, 128)
    V_H0C1 = slice(128, 193)  # h0 dh128:192 + ones0 @192 -> denom row 64
    V_H1C1 = slice(193, 258)  # h1 dh128:192 + ones1 @257 -> denom row 64
    V_H1C0 = slice(258, 386)
    VW = 386

    with tile.TileContext(nc) as tc:
        with (
            tc.tile_pool(name="const", bufs=1) as const,
            tc.tile_pool(name="atp", bufs=2) as atp,
            tc.tile_pool(name="ost", bufs=3) as ostp,
            tc.tile_pool(name="rcp", bufs=2) as rcp,
            tc.tile_pool(name="pps", bufs=2, space="PSUM") as pps,
            tc.tile_pool(name="scps", bufs=1, space="PSUM") as scps,
            tc.tile_pool(name="avps", bufs=1, space="PSUM") as avps,
        ):
            # ---- persistent SBUF tensors ----
            xT_sb = const.tile([P, KD, S], bf16, tag="xT_sb")
            wq_sb = const.tile([P, KD, HD], bf16, tag="wq_sb")
            wk_sb = const.tile([P, KD, HD], bf16, tag="wk_sb")
            wv_sb = const.tile([P, KD, HD], bf16, tag="wv_sb")
            wo_sb = const.tile([P, 3, D], bf16, tag="wo_sb")
            qt_sb = const.tile([P, 3, S], bf16, tag="qt_sb")
            kt_sb = const.tile([P, 3, S], bf16, tag="kt_sb")
            v_sb = const.tile([P, NK, VW], bf16, tag="v_sb")
            pt_sb = const.tile([P, 3, S], bf16, tag="pt_sb")
            ones1 = const.tile([1, P], bf16, tag="ones1")
            warm = const.tile([1, 1], fp32, tag="warm")

            # ---- input DMAs (split planes across engine queues) ----
            qs = [nc.sync, nc.scalar, nc.gpsimd]
            for k in range(KD):
                qs[k % len(qs)].dma_start(
                    xT_sb[:, k, :], xT[k * P : (k + 1) * P, :]
                )
            nc.sync.dma_start(
                wq_sb[:], wqT.rearrange("(ko ki) j -> ki ko j", ki=P)
            )
            nc.scalar.dma_start(
                wk_sb[:], wkT.rearrange("(ko ki) j -> ki ko j", ki=P)
            )
            nc.gpsimd.dma_start(
                wv_sb[:], wvT.rearrange("(ko ki) j -> ki ko j", ki=P)
            )
            for c in range(3):
                qs[c].dma_start(wo_sb[:, c, :], woS[c * P : (c + 1) * P, :])

            nc.vector.memset(ones1[:], 1.0)
            # prefetch the exp table while the PE does projections
            nc.scalar.activation(warm[:], ones1[0:1, 0:1], Exp)

            # ---- Q/K projections (transposed outputs, 3 full planes) ----
            for w_sb, o_sb in ((wq_sb, qt_sb), (wk_sb, kt_sb)):
                for c in range(3):
                    for nt in range(NQ):
                        ps = pps.tile([P, QB], fp32, tag="mm_ps")
                        for k in range(KD):
                            nc.tensor.matmul(
                                ps,
                                lhsT=w_sb[:, k, c * P : (c + 1) * P],
                                rhs=xT_sb[:, k, nt * QB : (nt + 1) * QB],
                                start=(k == 0),
                                stop=(k == KD - 1),
                            )
                        nc.vector.tensor_copy(
                            o_sb[:, c, nt * QB : (nt + 1) * QB], ps
                        )

            # ---- V projection (natural layout) + ones columns ----
            for m in range(MS):
                ps = pps.tile([P, QB], fp32, tag="mm_ps")
                for k in range(KD):
                    nc.tensor.matmul(
                        ps[:, 0:HD],
                        lhsT=xT_sb[:, k, m * P : (m + 1) * P],
                        rhs=wv_sb[:, k, :],
                        start=(k == 0),
                        stop=(k == KD - 1),
                    )
                nc.vector.tensor_copy(v_sb[:, m, 0:192], ps[:, 0:192])
                nc.vector.tensor_copy(v_sb[:, m, 193:257], ps[:, 192:256])
                nc.vector.tensor_copy(v_sb[:, m, 258:386], ps[:, 256:384])
                nc.vector.memset(v_sb[:, m, 192:193], 1.0)
                nc.vector.memset(v_sb[:, m, 257:258], 1.0)

            # ---- attention per q-block; out-proj deferred one block ----
            def out_proj(qj):
                for mi in range(4):
                    m = qj * 4 + mi
                    ost = ostp.tile([P, D], fp32, tag="ost")
                    for n in range(2):
                        ps = pps.tile([P, QB], fp32, tag="mm_ps")
                        for c in range(3):
                            nc.tensor.matmul(
                                ps[:, 0:384],
                                lhsT=pt_sb[:, c, m * P : (m + 1) * P],
                                rhs=wo_sb[:, c, n * 384 : (n + 1) * 384],
                                start=(c == 0),
                                stop=(c == 2),
                            )
                        nc.vector.tensor_copy(
                            ost[:, n * 384 : (n + 1) * 384], ps[:, 0:384]
                        )
                    nc.sync.dma_start(out[m * P : (m + 1) * P, :], ost[:])

            for qj in range(NQ):
                qsl = slice(qj * QB, (qj + 1) * QB)
                nk = 4 * qj + 4  # live key blocks (causal)
                at0 = atp.tile([P, NK, QB], bf16, tag="at0")
                at1 = atp.tile([P, NK, QB], bf16, tag="at1")
                for ki in range(nk):
                    ksl = slice(ki * KB, (ki + 1) * KB)
                    ps0 = scps.tile([P, QB], fp32, tag="sc_h0")
                    ps1 = scps.tile([P, QB], fp32, tag="sc_h1")
                    # full-plane matmuls (128 contraction rows)
                    nc.tensor.matmul(
                        ps0, lhsT=kt_sb[:, 0, ksl], rhs=qt_sb[:, 0, qsl],
                        start=True, stop=False,
                    )
                    nc.tensor.matmul(
                        ps1, lhsT=kt_sb[:, 1, ksl], rhs=qt_sb[:, 1, qsl],
                        start=True, stop=False,
                    )
                    # 64-row tails in disjoint row groups (concurrent)
                    nc.tensor.matmul(
                        ps0, lhsT=kt_sb[0:64, 2, ksl], rhs=qt_sb[0:64, 2, qsl],
                        start=False, stop=True,
                    )
                    nc.tensor.matmul(
                        ps1,
                        lhsT=kt_sb[64:128, 2, ksl],
                        rhs=qt_sb[64:128, 2, qsl],
                        start=False, stop=True,
                    )
                    d = ki - 4 * qj
                    for h, ps, at in ((0, ps0, at0), (1, ps1, at1)):
                        nc.scalar.activation(
                            at[:, ki, :], ps, Exp, scale=SCALE
                        )
                        if d >= 0:
                            # zero the non-causal part post-exp (Pool engine,
                            # off the PE/ACT critical path):
                            # keep iff q_local >= 128*d + k_local
                            nc.gpsimd.affine_select(
                                out=at[:, ki, :],
                                in_=at[:, ki, :],
                                compare_op=mybir.AluOpType.is_ge,
                                fill=0.0,
                                base=-128 * d,
                                pattern=[[1, QB]],
                                channel_multiplier=-1,
                            )
                # AV per head: c0 [128] + c1 [65] (with denominator row)
                psA = avps.tile([P, QB], fp32, tag="avA")
                psB = avps.tile([P, QB], fp32, tag="avB")
                psC = avps.tile([P, QB], fp32, tag="avC")
                psD = avps.tile([P, QB], fp32, tag="avD")
                for ki in range(nk):
                    nc.tensor.matmul(
                        psA, lhsT=v_sb[:, ki, V_H0C0], rhs=at0[:, ki, :],
                        start=(ki == 0), stop=(ki == nk - 1),
                    )
                    nc.tensor.matmul(
                        psC[0:65], lhsT=v_sb[:, ki, V_H0C1],
                        rhs=at0[:, ki, :],
                        start=(ki == 0), stop=(ki == nk - 1),
                    )
                for ki in range(nk):
                    nc.tensor.matmul(
                        psB, lhsT=v_sb[:, ki, V_H1C0], rhs=at1[:, ki, :],
                        start=(ki == 0), stop=(ki == nk - 1),
                    )
                    nc.tensor.matmul(
                        psD[0:65], lhsT=v_sb[:, ki, V_H1C1],
                        rhs=at1[:, ki, :],
                        start=(ki == 0), stop=(ki == nk - 1),
                    )
                for h, psc0, psc1, dnr, dhsl in (
                    (0, psA, psC, 64, slice(0, 64)),
                    (1, psB, psD, 64, slice(0, 64)),
                ):
                    rc = rcp.tile([1, QB], fp32, tag="rc")
                    rcb = rcp.tile([P, QB], fp32, tag="rcb")
                    nc.vector.reciprocal(rc, psc1[dnr : dnr + 1, :])
                    nc.gpsimd.partition_broadcast(rcb[:], rc[:])
                    nc.vector.tensor_mul(
                        pt_sb[:, h, qsl], psc0, rcb[:]
                    )
                    # upper 64 head dims land in plane 2: h0 -> partitions
                    # 0:64, h1 -> partitions 64:128 (partition-shifted write)
                    if h == 0:
                        nc.vector.tensor_mul(
                            pt_sb[0:64, 2, qsl], psc1[dhsl], rcb[0:64]
                        )
                    else:
                        nc.vector.tensor_mul(
                            pt_sb[64:128, 2, qsl], psc1[dhsl], rcb[64:128]
                        )
                if qj > 0:
                    out_proj(qj - 1)
            out_proj(NQ - 1)

    nc.compile()
    return nc


def _shard_inputs(x, Wq, Wk, Wv, Wo):
    bf = ml_dtypes.bfloat16
    in_maps = []
    for core in range(8):
        b, hp = core // 2, core % 2
        cols = slice(hp * HD, (hp + 1) * HD)
        in_maps.append(
            {
                "xT": np.ascontiguousarray(x[b].T).astype(bf),
                "wqT": np.ascontiguousarray(Wq[cols, :].T[:, PQ]).astype(bf),
                "wkT": np.ascontiguousarray(Wk[cols, :].T[:, PQ]).astype(bf),
                "wvT": np.ascontiguousarray(Wv[cols, :].T[:, PV]).astype(bf),
                "woS": np.ascontiguousarray(Wo[:, cols].T[PQ, :]).astype(bf),
            }
        )
    return in_maps


def _run(inputs, trace=False, **kw):
    from concourse.bass_utils import run_bass_kernel_spmd

    if "nc" not in _CACHED:
        _CACHED["nc"] = build_nc()
    nc = _CACHED["nc"]
    in_maps = _shard_inputs(
        np.asarray(inputs["x"], np.float32),
        np.asarray(inputs["Wq"], np.float32),
        np.asarray(inputs["Wk"], np.float32),
        np.asarray(inputs["Wv"], np.float32),
        np.asarray(inputs["Wo"], np.float32),
    )
    res = run_bass_kernel_spmd(
        nc, in_maps, core_ids=list(range(8)), trace=trace, **kw
    )
    parts = [np.asarray(r["out"], np.float32) for r in res.results]
    full = np.empty((B, S, D), np.float32)
    for b in range(B):
        full[b] = parts[2 * b] + parts[2 * b + 1]
    return full, res


def kernel(**inputs) -> np.ndarray:
    full, _ = _run(inputs, trace=False)
    return full
